# revision 31
# baseline (speedup 1.0000x reference)
"""Multi-head attention block on 8 NeuronCores (Trainium2, Bass/Tile).

Sharding: head-parallel tensor parallelism. Each core owns 2 of the 16
heads (a 128-wide slice of the projected feature dim). Per core:
  - Q/K/V projections for its feature slice, feature-major layout
    ([feature, token]); fp32r matmuls; outputs evacuated to bf16.
  - V is PE-transposed to token-major (bf16) with an appended ones
    column, so the attention-value matmul produces both the unnormalized
    output and the softmax denominator (row 64) in one accumulation.
  - Scores/exp/AV all bf16 operands (fp32 PSUM accumulation); softmax
    skips max-subtraction (bf16 exp has fp32-like range).
  - Output projection emits bf16 partials [1024, 4096]; the host sums
    the 8 partials in fp64 and adds bo exactly.
Schedule: a hand-interleaved emission program derived from the DMA
arrival timeline (the DMA device, ~360 GB/s aggregate, is co-critical
with the PE): the critical prefix loads only K(b0)+Q(b0,chunk0), score
groups of the first unit are woven between K-chunk projections as the
keys arrive, and all later projections / AV passes / output pieces are
placed where the PE would otherwise idle. Output-piece PSUM->bf16 casts
run on the DVE early and on the scalar engine for the last token chunk
(after the exp spine has drained).
"""

import sys

import numpy as np

if "/opt/trn_rl_repo" not in sys.path:
    sys.path.insert(0, "/opt/trn_rl_repo")

B = 2
S = 2048
D = 1024
H = 16
DH = 64
NCORES = 8
TOK = B * S  # 4096
FPC = D // NCORES  # features per core = 128
HPC = FPC // DH  # heads per core = 2
KD = D // 128  # contraction chunks for projections = 8
NTT = TOK // 128  # 128-token tiles = 32

_CACHE = {}


def _build(repeat=1):
    import concourse.bass as bass
    import concourse.mybir as mybir
    import concourse.tile as tile
    from concourse import bacc
    F32 = mybir.dt.float32
    F32R = mybir.dt.float32r
    BF16 = mybir.dt.bfloat16
    AF = mybir.ActivationFunctionType

    nc = bacc.Bacc()

    qT = nc.dram_tensor("qT", [D, TOK], F32, kind="ExternalInput")
    kT = nc.dram_tensor("kT", [D, TOK], F32, kind="ExternalInput")
    vT = nc.dram_tensor("vT", [D, TOK], F32, kind="ExternalInput")
    wqT = nc.dram_tensor("wqT", [D, FPC], F32, kind="ExternalInput")
    wkT = nc.dram_tensor("wkT", [D, FPC], F32, kind="ExternalInput")
    wvT = nc.dram_tensor("wvT", [D, FPC], F32, kind="ExternalInput")
    woT = nc.dram_tensor("woT", [FPC, D], BF16, kind="ExternalInput")
    bq = nc.dram_tensor("bq", [FPC, 1], F32, kind="ExternalInput")
    bk = nc.dram_tensor("bk", [FPC, 1], F32, kind="ExternalInput")
    bv = nc.dram_tensor("bv", [FPC, 1], F32, kind="ExternalInput")
    ident = nc.dram_tensor("ident", [128, 128], BF16, kind="ExternalInput")
    outT = nc.dram_tensor("outT", [D, TOK], BF16, kind="ExternalOutput")

    scale = 1.0 / np.sqrt(DH)

    # unit table: first all of batch 0 (qc-major, h inner), then batch 1
    # with h=1 first so the partition-shift staging DMA lands early.
    UNITS = [(0, h, qc) for qc in range(4) for h in range(HPC)]
    UNITS += [(1, 1 - i, qc) for qc in range(4) for i in range(2)]

    with tile.TileContext(nc) as tc:
        with tc.tile_pool(name="persist", bufs=1) as pp:
            QT = pp.tile([128, TOK], BF16)  # [feature, token]
            KT = pp.tile([128, TOK], BF16)
            V65 = pp.tile([128, NTT, HPC * 65], BF16)
            ATT = pp.tile([128, TOK], BF16)
            WO = pp.tile([128, D], BF16)
            WQ = pp.tile([128, KD, FPC], F32R)
            WK = pp.tile([128, KD, FPC], F32R)
            WV = pp.tile([128, KD, FPC], F32R)
            BQ = pp.tile([128, 1], F32)
            BK = pp.tile([128, 1], F32)
            BV = pp.tile([128, 1], F32)
            IDENT = pp.tile([128, 128], BF16)

            # Critical-prefix consts: Q weights first (the very first
            # projection is Q chunk 0), then K weights while Q0 streams.
            nc.sync.dma_start(
                out=WQ, in_=wqT.ap().rearrange("(c p) m -> p c m", p=128).bitcast(F32R)
            )
            nc.sync.dma_start(out=BQ, in_=bq.ap())
            nc.sync.dma_start(
                out=WK, in_=wkT.ap().rearrange("(c p) m -> p c m", p=128).bitcast(F32R)
            )
            nc.sync.dma_start(out=BK, in_=bk.ap())
            ACTWARM = pp.tile([128, 1], F32)
            nc.scalar.activation(ACTWARM[:, :], BK[:, :], AF.Exp)
            v65_4d = V65.rearrange("p t (h c) -> p t h c", h=HPC)

            def load_v_consts():
                nc.sync.dma_start(
                    out=WV,
                    in_=wvT.ap().rearrange("(c p) m -> p c m", p=128).bitcast(F32R),
                )
                nc.sync.dma_start(out=BV, in_=bv.ap())
                nc.sync.dma_start(out=IDENT, in_=ident.ap())
                nc.vector.memset(v65_4d[:, :, :, 64:65], 1.0)

            def load_wo():
                nc.sync.dma_start(out=WO, in_=woT.ap())

            for _rep in range(repeat):
                with tc.tile_pool(name="xin", bufs=8) as xpool, tc.tile_pool(
                    name="ps", bufs=1, space="PSUM"
                ) as pstool, tc.tile_pool(name="work", bufs=2) as wpool, \
                    tc.tile_pool(name="expT", bufs=5) as epool, \
                    tc.tile_pool(name="norm", bufs=2) as npool, \
                    tc.tile_pool(name="outsb", bufs=3) as opool:

                    inflight = {}
                    exts = {}

                    def load(kind, n):
                        """Issue the 4 xin DMAs for one 512-token chunk."""
                        src_ = {"q": qT, "k": kT, "v": vT}[kind]
                        src_r = (
                            src_.ap()
                            .rearrange("(c p) n -> p c n", p=128)
                            .bitcast(F32R)
                        )
                        ns = bass.ts(n, 512)
                        half = KD // 2
                        xins = []
                        for hh in range(2):
                            xin = xpool.tile(
                                [128, half, 512], F32R, tag="xin", name="xin"
                            )
                            for qtr in range(2):
                                sl = slice(2 * qtr, 2 * qtr + 2)
                                gsl = slice(
                                    hh * half + 2 * qtr, hh * half + 2 * qtr + 2
                                )
                                nc.sync.dma_start(
                                    out=xin[:, sl, :], in_=src_r[:, gsl, ns]
                                )
                            xins.append(xin)
                        inflight[(kind, n)] = xins

                    def projc(kind, n):
                        """Project one loaded 512-token chunk (feature-major)."""
                        wsb, bsb, dst = {
                            "q": (WQ, BQ, QT),
                            "k": (WK, BK, KT),
                            "v": (WV, BV, None),
                        }[kind]
                        xins = inflight.pop((kind, n))
                        ns = bass.ts(n, 512)
                        half = KD // 2
                        ps = pstool.tile([128, 512], F32, tag="pp", bufs=2, name="ps")
                        for c in range(KD):
                            nc.tensor.matmul(
                                ps[:, :],
                                wsb[:, c, :],
                                xins[c // half][:, c % half, :],
                                start=(c == 0),
                                stop=(c == KD - 1),
                            )
                        if dst is not None:
                            nc.vector.tensor_scalar_add(dst[:, ns], ps[:, :], bsb[:, :])
                        else:
                            vt = wpool.tile([128, 512], BF16, tag="vtmp", name="vt")
                            nc.vector.tensor_scalar_add(vt[:, :], ps[:, :], bsb[:, :])
                            for j in range(4):
                                tt = 4 * n + j
                                tp = pstool.tile(
                                    [128, 512], BF16, tag="pp", bufs=2, name="tp"
                                )
                                nc.tensor.transpose(
                                    tp[:, 0:128], vt[:, bass.ts(j, 128)], IDENT[:, :]
                                )
                                nc.vector.tensor_copy(
                                    v65_4d[:, tt, :, 0:64],
                                    tp[:, 0:128].rearrange("p (h c) -> p h c", h=HPC),
                                )

                    def sg(u, g):
                        """One score group (2 key tiles) + its exp."""
                        b, h, qc = UNITS[u]
                        if g == 0:
                            exts[u] = epool.tile(
                                [128, 16, 512], BF16, tag="expT", name="ex"
                            )
                        ex = exts[u]
                        exf = ex.rearrange("p k n -> p (k n)")
                        hs = slice(DH * h, DH * (h + 1))
                        qs = bass.ds(2048 * b + 512 * qc, 512)
                        sp = pstool.tile([128, 1024], F32, tag="sc", bufs=2, name="sp")
                        for j in range(2):
                            kt = 2 * g + j
                            ks = bass.ds(2048 * b + 128 * kt, 128)
                            nc.tensor.matmul(
                                sp[:, bass.ts(j, 512)],
                                KT[hs, ks],
                                QT[hs, qs],
                                start=True,
                                stop=True,
                            )
                        nc.scalar.activation(
                            exf[:, bass.ts(g, 1024)],
                            sp[:, :],
                            AF.Exp,
                            scale=float(scale),
                        )

                    def sgs(u, gs=range(8)):
                        for g in gs:
                            sg(u, g)

                    def av_open(u):
                        b, h, qc = UNITS[u]
                        avp = pstool.tile([65, 512], F32, tag="av", bufs=2, name="av")
                        return avp

                    def av_kts(u, avp, k0, k1):
                        b, h, qc = UNITS[u]
                        ex = exts[u]
                        for kt in range(k0, k1):
                            tt = 16 * b + kt
                            nc.tensor.matmul(
                                avp[:, :],
                                V65[:, tt, 65 * h : 65 * h + 65],
                                ex[:, kt, :],
                                start=(kt == 0),
                                stop=(kt == 15),
                            )

                    def av_norm(u, avp):
                        b, h, qc = UNITS[u]
                        exts.pop(u)
                        qs = bass.ds(2048 * b + 512 * qc, 512)
                        rec = npool.tile([1, 512], F32, tag="rec", name="rec")
                        nc.vector.reciprocal(rec[:, :], avp[64:65, :])
                        recb = npool.tile([64, 512], F32, tag="recb", name="recb")
                        nc.gpsimd.partition_broadcast(recb[:, :], rec[:, :])
                        if h == 0:
                            nc.vector.tensor_tensor(
                                ATT[0:64, qs], avp[0:64, :], recb[:, :],
                                mybir.AluOpType.mult,
                            )
                        else:
                            stage = npool.tile(
                                [64, 512], BF16, tag="stage", name="stage"
                            )
                            nc.vector.tensor_tensor(
                                stage[:, :], avp[0:64, :], recb[:, :],
                                mybir.AluOpType.mult,
                            )
                            nc.gpsimd.dma_start(out=ATT[64:128, qs], in_=stage[:, :])

                    def av(u):
                        avp = av_open(u)
                        av_kts(u, avp, 0, 16)
                        av_norm(u, avp)

                    def sgs_weave(u, wu):
                        """Scores of unit u with av matmuls of unit wu woven
                        between score groups (2 kt per group boundary)."""
                        avp = av_open(wu)
                        for g in range(8):
                            sg(u, g)
                            av_kts(wu, avp, 2 * g, 2 * g + 2)
                        av_norm(wu, avp)

                    def outp(t, jc, eng="dve"):
                        ts_ = bass.ts(t, 512)
                        op = pstool.tile(
                            [128, 512], F32, tag="pp", bufs=2, name="op"
                        )
                        nc.tensor.matmul(
                            op[:, :], WO[:, bass.ts(jc, 128)], ATT[:, ts_],
                            start=True, stop=True,
                        )
                        ob = opool.tile([128, 512], BF16, tag="ob", name="ob")
                        # cast + DMA-issue stay off the SP queue so xin loads
                        # are never head-of-line blocked behind an outT write
                        if eng == "act":
                            nc.scalar.copy(ob[:, :], op[:, :])
                            nc.scalar.dma_start(
                                out=outT[bass.ts(jc, 128), ts_], in_=ob[:, :]
                            )
                        else:
                            nc.vector.tensor_copy(ob[:, :], op[:, :])
                            nc.gpsimd.dma_start(
                                out=outT[bass.ts(jc, 128), ts_], in_=ob[:, :]
                            )

                    def outs(t, eng="dve"):
                        for jc in range(KD):
                            outp(t, jc, eng)

                    # ================= emission program =================
                    # Phase A: critical prefix — Q0, K0..K3 stream; unit 0's
                    # score groups woven between K-chunk projections.
                    load("q", 0)
                    load("k", 0)
                    load("k", 1)
                    projc("q", 0)
                    projc("k", 0)
                    load("k", 2)
                    sgs(0, range(0, 2))
                    projc("k", 1)
                    load("k", 3)
                    sgs(0, range(2, 4))
                    projc("k", 2)
                    load("v", 0)
                    if _rep == 0:
                        load_v_consts()
                    sgs(0, range(4, 6))
                    projc("k", 3)
                    load("q", 1)
                    sgs(0, range(6, 8))
                    sgs(1, range(0, 4))
                    load("v", 1)
                    if _rep == 0:
                        load_wo()
                    sgs(1, range(4, 8))

                    # Phase B: b0 ramp — remaining b0 chunks stream while
                    # units 2..7 run; avs start once V(b0) is resident.
                    projc("v", 0)
                    load("q", 2)
                    projc("q", 1)
                    sgs(2)
                    projc("v", 1)
                    load("v", 2)
                    sgs(3)
                    projc("q", 2)
                    load("v", 3)
                    projc("v", 2)
                    load("k", 4)
                    sgs(4)
                    projc("v", 3)
                    load("q", 3)
                    av(0)
                    av(1)
                    sgs(5)
                    projc("q", 3)
                    load("k", 5)
                    av(2)
                    sgs(6)
                    projc("k", 4)
                    load("k", 6)
                    av(3)
                    sgs(7)
                    projc("k", 5)
                    load("k", 7)
                    outs(0, "dve")

                    # Phase C: b1 ramp / b0 drain.
                    projc("k", 6)
                    load("q", 4)
                    av(4)
                    av(5)
                    projc("k", 7)
                    load("v", 4)
                    outs(1, "dve")
                    projc("q", 4)
                    load("v", 5)
                    av(6)
                    av(7)
                    sgs(8)
                    projc("v", 4)
                    load("q", 5)
                    sgs(9)
                    projc("v", 5)
                    load("v", 6)
                    projc("q", 5)
                    load("v", 7)
                    sgs(10)
                    projc("v", 6)
                    load("q", 6)
                    sgs(11)
                    projc("v", 7)
                    load("q", 7)

                    # Phase D: b1 steady — V(b1) resident, avs resume; outT
                    # pieces fill the now-free DMA device.
                    av(8)
                    projc("q", 6)
                    outs(2, "dve")
                    sgs(12)
                    av(9)
                    outs(3, "dve")
                    sgs(13)
                    projc("q", 7)
                    av(10)
                    outs(4, "dve")
                    sgs(14)
                    av(11)
                    outs(5, "dve")
                    sgs(15)
                    av(12)
                    av(13)
                    outs(6, "dve")
                    av(14)
                    av(15)
                    outs(7, "act")

    nc.compile()
    return nc


def _prep_inputs(q, k, v, wq, bq, wk, bk, wv, bv, wo, bo):
    import ml_dtypes

    bf16 = np.dtype(ml_dtypes.bfloat16)
    qT = np.ascontiguousarray(q.reshape(TOK, D).T).astype(np.float32)
    kT = np.ascontiguousarray(k.reshape(TOK, D).T).astype(np.float32)
    vT = np.ascontiguousarray(v.reshape(TOK, D).T).astype(np.float32)
    in_maps = []
    for c in range(NCORES):
        fs = slice(FPC * c, FPC * (c + 1))
        in_maps.append(
            {
                "qT": qT,
                "kT": kT,
                "vT": vT,
                "wqT": np.ascontiguousarray(wq[fs, :].T).astype(np.float32),
                "wkT": np.ascontiguousarray(wk[fs, :].T).astype(np.float32),
                "wvT": np.ascontiguousarray(wv[fs, :].T).astype(np.float32),
                "woT": np.ascontiguousarray(wo[:, fs].T).astype(bf16),
                "bq": bq[fs].reshape(FPC, 1).astype(np.float32),
                "bk": bk[fs].reshape(FPC, 1).astype(np.float32),
                "bv": bv[fs].reshape(FPC, 1).astype(np.float32),
                "ident": np.eye(128, dtype=np.float32).astype(bf16),
            }
        )
    return in_maps


def run(inputs, trace=False):
    """Run the SPMD kernel; returns (output [B,S,D] fp32, BassKernelResults)."""
    if "nc" not in _CACHE:
        _CACHE["nc"] = _build()
    nc = _CACHE["nc"]
    return _run_nc(nc, inputs, trace)


def _run_nc(nc, inputs, trace=False):
    from concourse.bass_utils import run_bass_kernel_spmd

    bo = np.asarray(inputs["bo"], np.float32)
    in_maps = _prep_inputs(
        np.asarray(inputs["q"], np.float32),
        np.asarray(inputs["k"], np.float32),
        np.asarray(inputs["v"], np.float32),
        np.asarray(inputs["wq"], np.float32),
        np.asarray(inputs["bq"], np.float32),
        np.asarray(inputs["wk"], np.float32),
        np.asarray(inputs["bk"], np.float32),
        np.asarray(inputs["wv"], np.float32),
        np.asarray(inputs["bv"], np.float32),
        np.asarray(inputs["wo"], np.float32),
        bo,
    )
    res = run_bass_kernel_spmd(nc, in_maps, list(range(NCORES)), trace=trace)
    acc = np.zeros((D, TOK), np.float64)
    for c in range(NCORES):
        acc += res.results[c]["outT"].astype(np.float64)
    out = (acc.T + bo[None, :]).reshape(B, S, D).astype(np.float32)
    return out, res


def kernel(**inputs):
    out, _ = run(inputs, trace=False)
    return out
